# revision 17
# baseline (speedup 1.0000x reference)
"""Trainium2 Bass kernel for nn_AdvancedVibLayer (damped driven oscillator
lattice with 3x3 depthwise-conv coupling, 6 fused timesteps).

Reference math (per channel c, per pixel):
    w  = softplus(omega); z = sigmoid(zeta); w2 = w*w
    6 steps of:  I = dwconv3x3(x);  v += DT*(-2zw*v - w2*x + f + I);  x += DT*v
    out = 0.5*v^2 + 0.5*w2*x^2

Device formulation: eliminate v via the second-order recurrence
    x_{s+1} = a2*x_s - a*x_{s-1} + x1 + DT^2*Koff(x_s)
    a = 1 - 2zw*DT,  a2 = 1 + a - DT^2*w2 + DT^2*k_center
    Koff = the 8 off-center taps of the depthwise 3x3 conv
    x0 = 0, x1 = DT^2*f;  v6 = (x6-x5)/DT;  E = 0.5*v6^2 + 0.5*w2*x6^2

Engine mapping (v4 — fp8 DoubleRow taps + continuous flipped sweep):
  - TensorE: the 8 off-center taps as 4 fp8 DoubleRow matmuls (e5m2
    diagonal pair-weights x e4m3 shadow of x, 0.5 cyc/row = 4x bf16
    throughput).  Each pair reads two spatial shifts of the fp8 shadow via
    one 3-dim AP (row-step-2 slice, or a hand-built overlapping stride-2
    col pair).  The -a*x_{s-1} and +x1 terms also ride the PE as fp16
    diagonal matmuls into the same PSUM group ((1-a)*x1 fused at s=2).
  - DVE: ONE scalar_tensor_tensor per 4-row psum group:
    x_{s+1} = a2*x_s + psum; energy combine; small fp8-shadow pieces.
  - ScalarE: x1 shadows, fp16 shadows, energy square, fp8 share.
  - GPSIMD: energy subtract/add, fp8 share.

Layout: per core one batch element (batch-parallel over 8 cores).
128 partitions = 64 channels x 2 H-halves.  Half 0 sweeps image rows
0..127 top-down; half 1 is stored ROW-FLIPPED (buffer row p = image row
255-p) so one instruction stream sweeps both halves "down" with uniform
addressing — its tap weights use k[c, 2-dy, dx] and its DMA rows run
reversed (negative stride).  Each half processes 133 rows (5 tail halo
rows past the middle, recomputed at shrinking depth), so the halo tax is
4% instead of the 31% of 4-way H tiling, and the whole kernel is ONE
pipelined wavefront: unit (s, c) of wave s+c handles 8-row chunk c of
stage s (0 = x1 shadows, 1-5 = recurrence steps, 6 = energy+store).

State lives in rolling row-windows (mod-window slots, all sizes
multiples of 8 so 8-aligned chunks never wrap mid-instruction).  fp32
x_{s-1} is never read (shadows feed the matmuls): XA carries x2/x4/x6,
XB x3/x5 in place.  The fp8 shadow window has one guard row at each end
mirroring the opposite edge (wrap-adjacent rows) and a permanent zero
row for the image top; W padded to 258 with zero cols 0,257.
"""

import sys

sys.path.insert(0, "/opt/trn_rl_repo")

import numpy as np

B, C, H, W = 8, 64, 256, 256
DT = 0.2
WP = W + 2          # padded width, zero cols at 0 and WP-1
GH = H // 2
HALO = 5
NCORES = 8
RT = GH + HALO      # 133 sweep rows per half

WIN_A = 56          # XA window (x2/x4/x6), rows
WIN_B = 40          # XB window (x3/x5)
WIN_F = 16          # raw-force landing window
WIN_8 = 56          # fp8 shadow window: must span all 5 live
                    # states (rows ~8*(wave-s), spread 48) + margin
WIN_RB = 40         # fp16 shadow window, per plane
WIN_CB = 56         # fp16 x1 window (init runs 1 wave ahead)

# off-center taps as 4 DoubleRow pairs; (dy, dx) in 0..2, center (1,1)
# folded into a2.  First three pairs differ by dy=2 rows (natural
# step-2 row slice); the last differs by dx=2 cols (hand-built AP).
PAIRS = [
    ((0, 0), (2, 0)),
    ((0, 1), (2, 1)),
    ((0, 2), (2, 2)),
    ((1, 0), (1, 2)),
]

_NC_CACHE = None


def _build_nc():
    from concourse import bacc, mybir
    from concourse.tile import TileContext

    f32 = mybir.dt.float32
    f16 = mybir.dt.float16
    f8e4 = mybir.dt.float8e4
    f8e5 = mybir.dt.float8e5
    Alu = mybir.AluOpType
    Act = mybir.ActivationFunctionType
    DR = mybir.MatmulPerfMode.DoubleRow

    nc = bacc.Bacc(
        "TRN2", target_bir_lowering=False, debug=False, num_devices=NCORES
    )
    force_d = nc.dram_tensor("force", [C, H, W], f32, kind="ExternalInput")
    wpair_d = nc.dram_tensor("wpair", [128, 4, 2, 128], f8e5,
                             kind="ExternalInput")
    wb_d = nc.dram_tensor("wb", [128, 3, 128], f16, kind="ExternalInput")
    coef_d = nc.dram_tensor("coef", [128, 3], f32, kind="ExternalInput")
    out_d = nc.dram_tensor("out", [C, H, W], f32, kind="ExternalOutput")

    NCH = -(-RT // 8)           # sweep chunks (17)

    def L(s):
        return RT - s + 1       # valid rows of state x_s  (L(1) = 133)

    with TileContext(nc) as tc:
        with tc.tile_pool(name="coefp", bufs=1) as coefp, \
             tc.tile_pool(name="wp", bufs=1) as wp, \
             tc.tile_pool(name="xf", bufs=1) as xfp, \
             tc.tile_pool(name="xa", bufs=1) as xap, \
             tc.tile_pool(name="xb", bufs=1) as xbp, \
             tc.tile_pool(name="x8", bufs=1) as x8p, \
             tc.tile_pool(name="xrb", bufs=1) as xrbp, \
             tc.tile_pool(name="xcb", bufs=1) as xcbp, \
             tc.tile_pool(name="ps", bufs=4, space="PSUM") as psp:
            coef = coefp.tile([128, 3], f32)
            nc.gpsimd.dma_start(coef[:], coef_d[:])
            a2 = coef[:, 0:1]        # a2
            a2p1 = coef[:, 1:2]      # a2 + 1   (step 1; x1 comes from XCB)
            esc = coef[:, 2:3]       # w*sqrt(0.5)
            WPAIR = wp.tile([128, 4, 2, 128], f8e5)
            WB = wp.tile([128, 3, 128], f16)
            nc.gpsimd.dma_start(WPAIR[:], wpair_d[:])
            nc.gpsimd.dma_start(WB[:], wb_d[:])
            w_nega = WB[:, 0]        # diag(-a)
            w_1ma = WB[:, 1]         # diag(1-a)
            w_one = WB[:, 2]         # diag(1)

            XF = xfp.tile([128, WIN_F, W], f32)          # raw force
            XA = xap.tile([128, WIN_A, WP], f32)         # x2/x4/x6
            XB = xbp.tile([128, WIN_B, WP], f32)         # x3/x5
            X8 = x8p.tile([128, WIN_8 + 3, WP], f8e4)    # zero+guards+win
            XRB = xrbp.tile([128, 2, WIN_RB, WP], f16)   # fp16 x_m
            XCB = xcbp.tile([128, WIN_CB, WP], f16)      # fp16 x1
            for Xz in (XA, XB):
                nc.gpsimd.memset(Xz[:, :, 0:1], 0.0)
                nc.gpsimd.memset(Xz[:, :, WP - 1:WP], 0.0)
            nc.gpsimd.memset(X8[:, :, 0:1], 0.0)
            nc.gpsimd.memset(X8[:, :, WP - 1:WP], 0.0)
            nc.gpsimd.memset(X8[:, 0:1, :], 0.0)   # image-top zero row

            def sA(r):
                return r % WIN_A

            def sB(r):
                return r % WIN_B

            def sF(r):
                return r % WIN_F

            def s8(r):
                # slot 0: permanent zero (image top); slot 1: low guard
                # (mirror of rows = -1 mod WIN_8); slots 2..WIN_8+1: window;
                # slot WIN_8+2: high guard (mirror of rows = 0 mod WIN_8)
                return 2 + (r % WIN_8)

            def sRB(r):
                return r % WIN_RB

            def sCB(r):
                return r % WIN_CB

            import itertools
            x8_eng = itertools.cycle(["pool", "act", "dve", "pool", "act"])
            x8i_eng = itertools.cycle(["act", "dve", "act", "pool"])

            def eng_of(name):
                return {"act": nc.scalar, "dve": nc.vector,
                        "pool": nc.gpsimd}[name]

            def x8_write(eng, dr0, n, src, r0):
                """copy n rows src[r0..] (slot space) -> X8 rows dr0.."""
                e = eng_of(eng)
                d = X8[:, dr0:dr0 + n, 1:W + 1]
                sl = src[:, r0:r0 + n, 1:W + 1]
                if eng == "act":
                    e.copy(d, sl)
                else:
                    e.tensor_copy(d, sl)

            def x8_copy(lo, hi, src, smap):
                """fp8 shadow of rows [lo, hi) of src.  Split at 8-row
                boundaries so neither side wraps; duplicate wrap-edge rows
                into the guard rows."""
                r = lo
                while r < hi:
                    n = min(hi, (r // 8 + 1) * 8) - r
                    x8_write(next(x8_eng), s8(r), n, src, smap(r))
                    for q in range(r, r + n):
                        if q % WIN_8 == WIN_8 - 1:
                            x8_write("dve", 1, 1, src, smap(q))
                        elif q % WIN_8 == 0 and q > 0:
                            x8_write("dve", WIN_8 + 2, 1, src, smap(q))
                    r += n

            def unit_rows(s, c):
                r0 = 8 * c
                re = RT if s == 0 else (L(s + 1) if s <= 5 else GH)
                if r0 >= re:
                    return None
                return (r0, min(8, re - r0))

            def issue_dma_in(c):
                # force chunk c (both halves; half 1 row-reversed) into XF
                u = unit_rows(0, c)
                if u is None:
                    return
                r0, nr = u
                nc.sync.dma_start(
                    XF[0:64, sF(r0):sF(r0) + nr, :],
                    force_d[:, r0:r0 + nr, :],
                )
                nc.sync.dma_start(
                    XF[64:128, sF(r0):sF(r0) + nr, :],
                    force_d[:, H - 1 - r0:H - 1 - r0 - nr:-1, :],
                )

            def emit_init(c):
                r0, nr = unit_rows(0, c)
                issue_dma_in(c + WIN_F // 8 - 1)
                # x1 = DT^2*f shadows straight from XF: fp16 (XCB) and fp8
                nc.scalar.mul(XCB[:, sCB(r0):sCB(r0) + nr, 1:W + 1],
                              XF[:, sF(r0):sF(r0) + nr, :], DT * DT)
                for g0 in range(0, nr, 4):
                    gn = min(4, nr - g0)
                    q = r0 + g0
                    eng = next(x8i_eng)
                    e = eng_of(eng)
                    d = X8[:, s8(q):s8(q) + gn, 1:W + 1]
                    sl = XF[:, sF(q):sF(q) + gn, :]
                    if eng == "act":
                        e.mul(d, sl, DT * DT)
                    else:
                        e.tensor_scalar_mul(d, sl, DT * DT)
                    for q2 in range(q, q + gn):
                        if q2 % WIN_8 == WIN_8 - 1:
                            nc.vector.tensor_scalar_mul(
                                X8[:, 1:2, 1:W + 1],
                                XF[:, sF(q2):sF(q2) + 1, :], DT * DT)
                        elif q2 % WIN_8 == 0 and q2 > 0:
                            nc.vector.tensor_scalar_mul(
                                X8[:, WIN_8 + 2:WIN_8 + 3, 1:W + 1],
                                XF[:, sF(q2):sF(q2) + 1, :], DT * DT)

            bufs = {1: XA, 2: XB, 3: XA, 4: XB, 5: XA}
            smaps = {1: sA, 2: sB, 3: sA, 4: sB, 5: sA}

            def emit_step(s, c):
                r0, nr = unit_rows(s, c)
                Xn, sn = bufs[s], smaps[s]
                rbw = XRB[:, (s + 1) % 2]
                rbr = XRB[:, (s - 1) % 2]
                last_unit = unit_rows(s, c + 1) is None
                for g0 in range(0, nr, 4):
                    gn = min(4, nr - g0)
                    a = r0 + g0
                    psum = psp.tile([128, 4, 256], f32)
                    for k in range(gn):
                        row = a + k
                        po = psum[:, k:k + 1, :]
                        z = s8(row)
                        for j, (ta, tb) in enumerate(PAIRS):
                            if ta[0] != tb[0]:
                                dx = ta[1]
                                if row == 0:
                                    # row -1 is the permanent zero slot 0
                                    mv = X8[:, 0:2, dx:dx + 256].copy()
                                    mv.ap[1] = [s8(1) * WP, 2]
                                else:
                                    mv = X8[:, z - 1:z + 2:2, dx:dx + 256]
                            else:
                                mv = X8[:, z:z + 2, 0:256].copy()
                                mv.ap[1] = [2, 2]
                            nc.tensor.matmul(
                                po, WPAIR[:, j], mv,
                                start=(j == 0),
                                stop=(j == 3 and s == 1),
                                perf_mode=DR)
                        if s == 2:
                            # (1-a)*x1 = -a*x1 + x1 in one matmul
                            nc.tensor.matmul(
                                po, w_1ma,
                                XCB[:, sCB(row):sCB(row) + 1, 1:W + 1],
                                start=False, stop=True)
                        elif s >= 3:
                            nc.tensor.matmul(
                                po, w_nega,
                                rbr[:, sRB(row):sRB(row) + 1, 1:W + 1],
                                start=False, stop=False)
                            nc.tensor.matmul(
                                po, w_one,
                                XCB[:, sCB(row):sCB(row) + 1, 1:W + 1],
                                start=False, stop=True)
                    # x_{s+1} = a2*x_s + psum (s=1: (a2+1)*x1 from XCB)
                    if s == 1:
                        cur, sc = XCB[:, sCB(a):sCB(a) + gn, 1:W + 1], a2p1
                    else:
                        pm = smaps[s - 1]
                        cur = bufs[s - 1][:, pm(a):pm(a) + gn, 1:W + 1]
                        sc = a2
                    nc.vector.scalar_tensor_tensor(
                        Xn[:, sn(a):sn(a) + gn, 1:W + 1], cur, sc,
                        psum[:, 0:gn, :], Alu.mult, Alu.add)
                    # fp8 shadow, fine-grained; row a+gn-1 deferred (still
                    # read as x_s by the next group/unit's taps)
                    if s <= 4:
                        lo = a - 1 if (c, g0) != (0, 0) else a
                        hi = a + gn if (last_unit and g0 + gn == nr) \
                            else a + gn - 1
                        if hi > lo:
                            x8_copy(lo, hi, Xn, sn)
                if DBG_S and s == DBG_S - 1:
                    qn = min(r0 + nr, GH) - r0
                    if qn > 0:
                        nc.sync.dma_start(
                            out_d[:, r0:r0 + qn, :],
                            Xn[0:64, sn(r0):sn(r0) + qn, 1:W + 1])
                        nc.sync.dma_start(
                            out_d[:, H - 1 - r0:H - 1 - r0 - qn:-1, :],
                            Xn[64:128, sn(r0):sn(r0) + qn, 1:W + 1])
                # fp16 shadow (read by step s+2's -a matmul, exact rows)
                if s <= 3:
                    nc.scalar.copy(
                        rbw[:, sRB(r0):sRB(r0) + nr, 1:W + 1],
                        Xn[:, sn(r0):sn(r0) + nr, 1:W + 1])

            import os
            DBG_S = int(os.environ.get("DBG_S", "0"))

            def emit_energy(c):
                q0, nq = unit_rows(6, c)
                if DBG_S:
                    return
                for g0 in range(0, nq, 4):
                    gn = min(4, nq - g0)
                    a = q0 + g0
                    d = XB[:, sB(a):sB(a) + gn, 1:W + 1]
                    x6 = XA[:, sA(a):sA(a) + gn, 1:W + 1]
                    nc.gpsimd.tensor_sub(d, x6, d)
                    nc.vector.scalar_tensor_tensor(
                        d, d, 0.5 / (DT * DT), d, Alu.mult, Alu.mult)
                    nc.scalar.activation(x6, x6, Act.Square, scale=esc)
                    nc.gpsimd.tensor_add(d, d, x6)
                nc.sync.dma_start(
                    out_d[:, q0:q0 + nq, :],
                    XB[0:64, sB(q0):sB(q0) + nq, 1:W + 1],
                )
                nc.sync.dma_start(
                    out_d[:, H - 1 - q0:H - 1 - q0 - nq:-1, :],
                    XB[64:128, sB(q0):sB(q0) + nq, 1:W + 1],
                )

            for c in range(WIN_F // 8 - 1):
                issue_dma_in(c)
            # init (s=0) runs one wave ahead of its step-1 consumer so the
            # x1-shadow ACT ops are off the per-wave critical chain
            maxw = 7 + NCH
            for w in range(maxw + 1):
                for s in range(0, 7):
                    c = w if s == 0 else w - s - 1
                    if c < 0 or unit_rows(s, c) is None:
                        continue
                    if s == 0:
                        emit_init(c)
                    elif s <= 5:
                        emit_step(s, c)
                    else:
                        emit_energy(c)
    nc.compile()
    return nc


def _host_coeffs(coupling_w, omega, zeta):
    om = np.asarray(omega, np.float64)[0, :, 0, 0]
    ze = np.asarray(zeta, np.float64)[0, :, 0, 0]
    w = np.logaddexp(0.0, om)
    z = 1.0 / (1.0 + np.exp(-ze))
    w2 = w * w
    a = 1.0 - 2.0 * z * w * DT
    alpha = 1.0 + a - DT * DT * w2
    k = np.asarray(coupling_w, np.float64)[:, 0, :, :]  # [C,3,3]
    return w2, a, alpha, k


def _device_tables(coupling_w, omega, zeta):
    import ml_dtypes
    w2, a, alpha, k = _host_coeffs(coupling_w, omega, zeta)
    a2 = alpha + DT * DT * k[:, 1, 1]
    coef64 = np.stack([a2, a2 + 1.0, np.sqrt(0.5 * w2)], axis=1)
    coef = np.tile(coef64.astype(np.float32), (2, 1))  # [128, 3]

    p = np.arange(128)
    c = p % 64
    wpair = np.zeros((128, 4, 2, 128), np.float32)
    for j, pair in enumerate(PAIRS):
        for i, (dy, dx) in enumerate(pair):
            # half 1 (partitions 64..127) is stored row-flipped: dy mirrors
            dyv = np.where(p < 64, dy, 2 - dy)
            wpair[p, j, i, p] = DT * DT * k[c, dyv, dx]
    wpair = wpair.astype(ml_dtypes.float8_e5m2)

    av = np.tile(a, 2)
    wb = np.zeros((128, 3, 128), np.float32)
    wb[p, 0, p] = -av
    wb[p, 1, p] = 1.0 - av
    wb[p, 2, p] = 1.0
    wb = wb.astype(np.float16)
    return {"wpair": wpair, "wb": wb, "coef": coef}


def kernel(force, coupling_w, omega, zeta):
    global _NC_CACHE
    from concourse.bass_utils import run_bass_kernel_spmd

    force = np.ascontiguousarray(np.asarray(force, np.float32))
    tables = _device_tables(coupling_w, omega, zeta)
    if _NC_CACHE is None:
        _NC_CACHE = _build_nc()
    nc = _NC_CACHE
    in_maps = [
        {"force": force[kk], **tables}
        for kk in range(NCORES)
    ]
    res = run_bass_kernel_spmd(nc, in_maps, list(range(NCORES)))
    return np.stack([res.results[kk]["out"] for kk in range(NCORES)], axis=0)


# revision 19
# speedup vs baseline: 1.0079x; 1.0079x over previous
"""Trainium2 Bass kernel for nn_AdvancedVibLayer (damped driven oscillator
lattice with 3x3 depthwise-conv coupling, 6 fused timesteps).

Reference math (per channel c, per pixel):
    w  = softplus(omega); z = sigmoid(zeta); w2 = w*w
    6 steps of:  I = dwconv3x3(x);  v += DT*(-2zw*v - w2*x + f + I);  x += DT*v
    out = 0.5*v^2 + 0.5*w2*x^2

Device formulation: eliminate v via the second-order recurrence
    x_{s+1} = a2*x_s - a*x_{s-1} + x1 + DT^2*Koff(x_s)
    a = 1 - 2zw*DT,  a2 = 1 + a - DT^2*w2 + DT^2*k_center
    Koff = the 8 off-center taps of the depthwise 3x3 conv
    x0 = 0, x1 = DT^2*f;  v6 = (x6-x5)/DT;  E = 0.5*v6^2 + 0.5*w2*x6^2

Engine mapping (v4 — fp8 DoubleRow taps + continuous flipped sweep):
  - TensorE: the 8 off-center taps as 4 fp8 DoubleRow matmuls (e5m2
    diagonal pair-weights x e4m3 shadow of x, 0.5 cyc/row = 4x bf16
    throughput).  Each pair reads two spatial shifts of the fp8 shadow via
    one 3-dim AP (row-step-2 slice, or a hand-built overlapping stride-2
    col pair).  The -a*x_{s-1} and +x1 terms also ride the PE as fp16
    diagonal matmuls into the same PSUM group ((1-a)*x1 fused at s=2).
  - DVE: ONE scalar_tensor_tensor per 4-row psum group:
    x_{s+1} = a2*x_s + psum; energy combine; small fp8-shadow pieces.
  - ScalarE: x1 shadows, fp16 shadows, energy square, fp8 share.
  - GPSIMD: energy subtract/add, fp8 share.

Layout: per core one batch element (batch-parallel over 8 cores).
128 partitions = 64 channels x 2 H-halves.  Half 0 sweeps image rows
0..127 top-down; half 1 is stored ROW-FLIPPED (buffer row p = image row
255-p) so one instruction stream sweeps both halves "down" with uniform
addressing — its tap weights use k[c, 2-dy, dx] and its DMA rows run
reversed (negative stride).  Each half processes 133 rows (5 tail halo
rows past the middle, recomputed at shrinking depth), so the halo tax is
4% instead of the 31% of 4-way H tiling, and the whole kernel is ONE
pipelined wavefront: unit (s, c) of wave s+c handles 8-row chunk c of
stage s (0 = x1 shadows, 1-5 = recurrence steps, 6 = energy+store).

State lives in rolling row-windows (mod-window slots, all sizes
multiples of 8 so 8-aligned chunks never wrap mid-instruction).  fp32
x_{s-1} is never read (shadows feed the matmuls): XA carries x2/x4/x6,
XB x3/x5 in place.  The fp8 shadow window has one guard row at each end
mirroring the opposite edge (wrap-adjacent rows) and a permanent zero
row for the image top; W padded to 258 with zero cols 0,257.
"""

import sys

sys.path.insert(0, "/opt/trn_rl_repo")

import numpy as np

B, C, H, W = 8, 64, 256, 256
DT = 0.2
WP = W + 2          # padded width, zero cols at 0 and WP-1
GH = H // 2
HALO = 5
NCORES = 8
RT = GH + HALO      # 133 sweep rows per half

WIN_A = 56          # XA window (x2/x4/x6), rows
WIN_B = 48          # XB window (x3/x5)
WIN_F = 16          # raw-force landing window
WIN_8 = 56          # fp8 shadow window: must span all 5 live
                    # states (rows ~8*(wave-s), spread 48) + margin
WIN_RB = 40         # fp16 shadow window, per plane
WIN_CB = 48         # fp16 x1 window

# off-center taps as 4 DoubleRow pairs; (dy, dx) in 0..2, center (1,1)
# folded into a2.  First three pairs differ by dy=2 rows (natural
# step-2 row slice); the last differs by dx=2 cols (hand-built AP).
PAIRS = [
    ((0, 0), (2, 0)),
    ((0, 1), (2, 1)),
    ((0, 2), (2, 2)),
    ((1, 0), (1, 2)),
]

_NC_CACHE = None


def _build_nc():
    from concourse import bacc, mybir
    from concourse.tile import TileContext

    f32 = mybir.dt.float32
    f16 = mybir.dt.float16
    f8e4 = mybir.dt.float8e4
    f8e5 = mybir.dt.float8e5
    Alu = mybir.AluOpType
    Act = mybir.ActivationFunctionType
    DR = mybir.MatmulPerfMode.DoubleRow

    nc = bacc.Bacc(
        "TRN2", target_bir_lowering=False, debug=False, num_devices=NCORES
    )
    force_d = nc.dram_tensor("force", [C, H, W], f32, kind="ExternalInput")
    wpair_d = nc.dram_tensor("wpair", [128, 4, 2, 128], f8e5,
                             kind="ExternalInput")
    wb_d = nc.dram_tensor("wb", [128, 3, 128], f16, kind="ExternalInput")
    coef_d = nc.dram_tensor("coef", [128, 3], f32, kind="ExternalInput")
    out_d = nc.dram_tensor("out", [C, H, W], f32, kind="ExternalOutput")

    NCH = -(-RT // 8)           # sweep chunks (17)

    def L(s):
        return RT - s + 1       # valid rows of state x_s  (L(1) = 133)

    with TileContext(nc) as tc:
        with tc.tile_pool(name="coefp", bufs=1) as coefp, \
             tc.tile_pool(name="wp", bufs=1) as wp, \
             tc.tile_pool(name="xf", bufs=1) as xfp, \
             tc.tile_pool(name="xa", bufs=1) as xap, \
             tc.tile_pool(name="xb", bufs=1) as xbp, \
             tc.tile_pool(name="x8", bufs=1) as x8p, \
             tc.tile_pool(name="xrb", bufs=1) as xrbp, \
             tc.tile_pool(name="xcb", bufs=1) as xcbp, \
             tc.tile_pool(name="ps", bufs=4, space="PSUM") as psp:
            coef = coefp.tile([128, 3], f32)
            nc.gpsimd.dma_start(coef[:], coef_d[:])
            a2 = coef[:, 0:1]        # a2
            a2p1 = coef[:, 1:2]      # a2 + 1   (step 1; x1 comes from XCB)
            esc = coef[:, 2:3]       # w*sqrt(0.5)
            WPAIR = wp.tile([128, 4, 2, 128], f8e5)
            WB = wp.tile([128, 3, 128], f16)
            nc.gpsimd.dma_start(WPAIR[:], wpair_d[:])
            nc.gpsimd.dma_start(WB[:], wb_d[:])
            w_nega = WB[:, 0]        # diag(-a)
            w_1ma = WB[:, 1]         # diag(1-a)
            w_one = WB[:, 2]         # diag(1)

            XF = xfp.tile([128, WIN_F, W], f32)          # raw force
            XA = xap.tile([128, WIN_A, WP], f32)         # x2/x4/x6
            XB = xbp.tile([128, WIN_B, WP], f32)         # x3/x5
            X8 = x8p.tile([128, WIN_8 + 3, WP], f8e4)    # zero+guards+win
            XRB = xrbp.tile([128, 2, WIN_RB, WP], f16)   # fp16 x_m
            XCB = xcbp.tile([128, WIN_CB, WP], f16)      # fp16 x1
            for Xz in (XA, XB):
                nc.gpsimd.memset(Xz[:, :, 0:1], 0.0)
                nc.gpsimd.memset(Xz[:, :, WP - 1:WP], 0.0)
            nc.gpsimd.memset(X8[:, :, 0:1], 0.0)
            nc.gpsimd.memset(X8[:, :, WP - 1:WP], 0.0)
            nc.gpsimd.memset(X8[:, 0:1, :], 0.0)   # image-top zero row

            def sA(r):
                return r % WIN_A

            def sB(r):
                return r % WIN_B

            def sF(r):
                return r % WIN_F

            def s8(r):
                # slot 0: permanent zero (image top); slot 1: low guard
                # (mirror of rows = -1 mod WIN_8); slots 2..WIN_8+1: window;
                # slot WIN_8+2: high guard (mirror of rows = 0 mod WIN_8)
                return 2 + (r % WIN_8)

            def sRB(r):
                return r % WIN_RB

            def sCB(r):
                return r % WIN_CB

            import itertools
            x8_eng = itertools.cycle(["act", "dve", "act", "pool", "act"])
            x8i_eng = itertools.cycle(["act", "dve"])

            def eng_of(name):
                return {"act": nc.scalar, "dve": nc.vector,
                        "pool": nc.gpsimd}[name]

            def x8_write(eng, dr0, n, src, r0):
                """copy n rows src[r0..] (slot space) -> X8 rows dr0.."""
                e = eng_of(eng)
                d = X8[:, dr0:dr0 + n, 1:W + 1]
                sl = src[:, r0:r0 + n, 1:W + 1]
                if eng == "act":
                    e.copy(d, sl)
                else:
                    e.tensor_copy(d, sl)

            def x8_copy(lo, hi, src, smap):
                """fp8 shadow of rows [lo, hi) of src.  Split at 8-row
                boundaries so neither side wraps; duplicate wrap-edge rows
                into the guard rows."""
                r = lo
                while r < hi:
                    n = min(hi, (r // 8 + 1) * 8) - r
                    x8_write(next(x8_eng), s8(r), n, src, smap(r))
                    for q in range(r, r + n):
                        if q % WIN_8 == WIN_8 - 1:
                            x8_write("dve", 1, 1, src, smap(q))
                        elif q % WIN_8 == 0 and q > 0:
                            x8_write("dve", WIN_8 + 2, 1, src, smap(q))
                    r += n

            def unit_rows(s, c):
                r0 = 8 * c
                re = RT if s == 0 else (L(s + 1) if s <= 5 else GH)
                if r0 >= re:
                    return None
                return (r0, min(8, re - r0))

            def issue_dma_in(c):
                # force chunk c (both halves; half 1 row-reversed) into XF
                u = unit_rows(0, c)
                if u is None:
                    return
                r0, nr = u
                nc.sync.dma_start(
                    XF[0:64, sF(r0):sF(r0) + nr, :],
                    force_d[:, r0:r0 + nr, :],
                )
                nc.sync.dma_start(
                    XF[64:128, sF(r0):sF(r0) + nr, :],
                    force_d[:, H - 1 - r0:H - 1 - r0 - nr:-1, :],
                )

            def emit_init(c):
                r0, nr = unit_rows(0, c)
                issue_dma_in(c + WIN_F // 8 - 1)
                # x1 = DT^2*f shadows straight from XF: fp16 (XCB) and fp8
                nc.scalar.mul(XCB[:, sCB(r0):sCB(r0) + nr, 1:W + 1],
                              XF[:, sF(r0):sF(r0) + nr, :], DT * DT)
                for g0 in range(0, nr, 4):
                    gn = min(4, nr - g0)
                    q = r0 + g0
                    eng = next(x8i_eng)
                    e = eng_of(eng)
                    d = X8[:, s8(q):s8(q) + gn, 1:W + 1]
                    sl = XF[:, sF(q):sF(q) + gn, :]
                    if eng == "act":
                        e.mul(d, sl, DT * DT)
                    else:
                        e.tensor_scalar_mul(d, sl, DT * DT)
                    for q2 in range(q, q + gn):
                        if q2 % WIN_8 == WIN_8 - 1:
                            nc.vector.tensor_scalar_mul(
                                X8[:, 1:2, 1:W + 1],
                                XF[:, sF(q2):sF(q2) + 1, :], DT * DT)
                        elif q2 % WIN_8 == 0 and q2 > 0:
                            nc.vector.tensor_scalar_mul(
                                X8[:, WIN_8 + 2:WIN_8 + 3, 1:W + 1],
                                XF[:, sF(q2):sF(q2) + 1, :], DT * DT)

            bufs = {1: XA, 2: XB, 3: XA, 4: XB, 5: XA}
            smaps = {1: sA, 2: sB, 3: sA, 4: sB, 5: sA}

            def emit_step(s, c):
                r0, nr = unit_rows(s, c)
                Xn, sn = bufs[s], smaps[s]
                rbw = XRB[:, (s + 1) % 2]
                rbr = XRB[:, (s - 1) % 2]
                last_unit = unit_rows(s, c + 1) is None
                for g0 in range(0, nr, 4):
                    gn = min(4, nr - g0)
                    a = r0 + g0
                    psum = psp.tile([128, 4, 256], f32)
                    for k in range(gn):
                        row = a + k
                        po = psum[:, k:k + 1, :]
                        z = s8(row)
                        for j, (ta, tb) in enumerate(PAIRS):
                            if ta[0] != tb[0]:
                                dx = ta[1]
                                if row == 0:
                                    # row -1 is the permanent zero slot 0
                                    mv = X8[:, 0:2, dx:dx + 256].copy()
                                    mv.ap[1] = [s8(1) * WP, 2]
                                else:
                                    mv = X8[:, z - 1:z + 2:2, dx:dx + 256]
                            else:
                                mv = X8[:, z:z + 2, 0:256].copy()
                                mv.ap[1] = [2, 2]
                            nc.tensor.matmul(
                                po, WPAIR[:, j], mv,
                                start=(j == 0),
                                stop=(j == 3 and s == 1),
                                perf_mode=DR)
                        if s == 2:
                            # (1-a)*x1 = -a*x1 + x1 in one matmul
                            nc.tensor.matmul(
                                po, w_1ma,
                                XCB[:, sCB(row):sCB(row) + 1, 1:W + 1],
                                start=False, stop=True)
                        elif s >= 3:
                            nc.tensor.matmul(
                                po, w_nega,
                                rbr[:, sRB(row):sRB(row) + 1, 1:W + 1],
                                start=False, stop=False)
                            nc.tensor.matmul(
                                po, w_one,
                                XCB[:, sCB(row):sCB(row) + 1, 1:W + 1],
                                start=False, stop=True)
                    # x_{s+1} = a2*x_s + psum (s=1: (a2+1)*x1 from XCB)
                    if s == 1:
                        cur, sc = XCB[:, sCB(a):sCB(a) + gn, 1:W + 1], a2p1
                    else:
                        pm = smaps[s - 1]
                        cur = bufs[s - 1][:, pm(a):pm(a) + gn, 1:W + 1]
                        sc = a2
                    nc.vector.scalar_tensor_tensor(
                        Xn[:, sn(a):sn(a) + gn, 1:W + 1], cur, sc,
                        psum[:, 0:gn, :], Alu.mult, Alu.add)
                    # fp8 shadow, fine-grained; row a+gn-1 deferred (still
                    # read as x_s by the next group/unit's taps)
                    if s <= 4:
                        lo = a - 1 if (c, g0) != (0, 0) else a
                        hi = a + gn if (last_unit and g0 + gn == nr) \
                            else a + gn - 1
                        if hi > lo:
                            x8_copy(lo, hi, Xn, sn)
                if DBG_S and s == DBG_S - 1:
                    qn = min(r0 + nr, GH) - r0
                    if qn > 0:
                        nc.sync.dma_start(
                            out_d[:, r0:r0 + qn, :],
                            Xn[0:64, sn(r0):sn(r0) + qn, 1:W + 1])
                        nc.sync.dma_start(
                            out_d[:, H - 1 - r0:H - 1 - r0 - qn:-1, :],
                            Xn[64:128, sn(r0):sn(r0) + qn, 1:W + 1])
                # fp16 shadow (read by step s+2's -a matmul, exact rows)
                if s <= 3:
                    nc.scalar.copy(
                        rbw[:, sRB(r0):sRB(r0) + nr, 1:W + 1],
                        Xn[:, sn(r0):sn(r0) + nr, 1:W + 1])

            import os
            DBG_S = int(os.environ.get("DBG_S", "0"))

            def emit_energy(c):
                q0, nq = unit_rows(6, c)
                if DBG_S:
                    return
                for g0 in range(0, nq, 4):
                    gn = min(4, nq - g0)
                    a = q0 + g0
                    d = XB[:, sB(a):sB(a) + gn, 1:W + 1]
                    x6 = XA[:, sA(a):sA(a) + gn, 1:W + 1]
                    nc.gpsimd.tensor_sub(d, x6, d)
                    nc.vector.scalar_tensor_tensor(
                        d, d, 0.5 / (DT * DT), d, Alu.mult, Alu.mult)
                    nc.scalar.activation(x6, x6, Act.Square, scale=esc)
                    nc.gpsimd.tensor_add(d, d, x6)
                nc.sync.dma_start(
                    out_d[:, q0:q0 + nq, :],
                    XB[0:64, sB(q0):sB(q0) + nq, 1:W + 1],
                )
                nc.sync.dma_start(
                    out_d[:, H - 1 - q0:H - 1 - q0 - nq:-1, :],
                    XB[64:128, sB(q0):sB(q0) + nq, 1:W + 1],
                )

            for c in range(WIN_F // 8 - 1):
                issue_dma_in(c)
            maxw = 6 + NCH
            for w in range(maxw + 1):
                for s in range(0, 7):
                    c = w - s
                    if c < 0 or unit_rows(s, c) is None:
                        continue
                    if s == 0:
                        emit_init(c)
                    elif s <= 5:
                        emit_step(s, c)
                    else:
                        emit_energy(c)
    nc.compile()
    return nc


def _host_coeffs(coupling_w, omega, zeta):
    om = np.asarray(omega, np.float64)[0, :, 0, 0]
    ze = np.asarray(zeta, np.float64)[0, :, 0, 0]
    w = np.logaddexp(0.0, om)
    z = 1.0 / (1.0 + np.exp(-ze))
    w2 = w * w
    a = 1.0 - 2.0 * z * w * DT
    alpha = 1.0 + a - DT * DT * w2
    k = np.asarray(coupling_w, np.float64)[:, 0, :, :]  # [C,3,3]
    return w2, a, alpha, k


def _device_tables(coupling_w, omega, zeta):
    import ml_dtypes
    w2, a, alpha, k = _host_coeffs(coupling_w, omega, zeta)
    a2 = alpha + DT * DT * k[:, 1, 1]
    coef64 = np.stack([a2, a2 + 1.0, np.sqrt(0.5 * w2)], axis=1)
    coef = np.tile(coef64.astype(np.float32), (2, 1))  # [128, 3]

    p = np.arange(128)
    c = p % 64
    wpair = np.zeros((128, 4, 2, 128), np.float32)
    for j, pair in enumerate(PAIRS):
        for i, (dy, dx) in enumerate(pair):
            # half 1 (partitions 64..127) is stored row-flipped: dy mirrors
            dyv = np.where(p < 64, dy, 2 - dy)
            wpair[p, j, i, p] = DT * DT * k[c, dyv, dx]
    wpair = wpair.astype(ml_dtypes.float8_e5m2)

    av = np.tile(a, 2)
    wb = np.zeros((128, 3, 128), np.float32)
    wb[p, 0, p] = -av
    wb[p, 1, p] = 1.0 - av
    wb[p, 2, p] = 1.0
    wb = wb.astype(np.float16)
    return {"wpair": wpair, "wb": wb, "coef": coef}


def kernel(force, coupling_w, omega, zeta):
    global _NC_CACHE
    from concourse.bass_utils import run_bass_kernel_spmd

    force = np.ascontiguousarray(np.asarray(force, np.float32))
    tables = _device_tables(coupling_w, omega, zeta)
    if _NC_CACHE is None:
        _NC_CACHE = _build_nc()
    nc = _NC_CACHE
    in_maps = [
        {"force": force[kk], **tables}
        for kk in range(NCORES)
    ]
    res = run_bass_kernel_spmd(nc, in_maps, list(range(NCORES)))
    return np.stack([res.results[kk]["out"] for kk in range(NCORES)], axis=0)
